# revision 11
# baseline (speedup 1.0000x reference)
"""kNN (k=16) + grouped 3->64->64->64 MLP + neighbor max-pool on 8 TRN2 cores.

Pipeline (device does all O(N^2) compute, selection, and MLP flops):
  L1 : S[q,j] = 2<xq,xj> - |xj|^2 on PE (self is always row max); chunk-16 max;
       top-24 chunk ids per query via max8/max_index/match_replace rounds.
  host: gather the 24*16=384 candidate coords per query (index routing only).
  L2A: exact squared dists in reference fp32 arithmetic on the 384-wide
       compacted domain; exact top-17 (slot 0 = self) -> local indices.
  host: map local->global indices, gather the 16 neighbor coords.
  L2B: relative coords via matmul-folded subtract, packed 2-point 3-layer MLP
       on PE, max-pool over the 16 neighbors.

Sharding: core c handles batch c//2, query half c%2 (2048 queries each).
"""
import sys
import numpy as np

sys.path.insert(0, "/opt/trn_rl_repo")

import jax
import numpy as _np
from jax.sharding import Mesh, PartitionSpec
from jax.experimental.shard_map import shard_map

import concourse.bacc as bacc
import concourse.mybir as mybir
import concourse.tile as tile
from concourse import bass2jax
from concourse.bass2jax import _bass_exec_p, install_neuronx_cc_hook

F32 = mybir.dt.float32
U16 = mybir.dt.uint16
AX = mybir.AxisListType
OP = mybir.AluOpType
AF = mybir.ActivationFunctionType

B, N, C, K = 4, 4096, 64, 16
KK = K + 1              # 17
CH = 16                 # chunk size for the selection hierarchy
NCH = N // CH           # 256
NSEL = 24               # chunks kept per query (>= 17 guarantee + tie slack)
W = NSEL * CH           # 384 candidate superset per query
NQ = 2048               # queries per core
NBLK = NQ // 128        # 16
NEG = -1.0e30
NCORES = 8

_progs = {}


def _rounds(nc, sp, vals, out_ids, tag):
    """3x (max8 -> max_index -> match_replace) producing 24 ids, mutating vals."""
    for r in range(3):
        m8 = sp.tile([128, 8], F32, tag=f"m8{tag}", name=f"m8{tag}_{r}_{id(vals)}")
        nc.vector.max(out=m8[:], in_=vals)
        nc.vector.max_index(out=out_ids[:, r * 8:(r + 1) * 8], in_max=m8[:],
                            in_values=vals)
        if r < 2:
            nc.vector.match_replace(out=vals, in_to_replace=m8[:], in_values=vals,
                                    imm_value=NEG)


def _build_l1(repeat=1):
    nc = bacc.Bacc("TRN2", target_bir_lowering=False, debug=False,
                   num_devices=NCORES)
    xyzT_d = nc.dram_tensor("xyzT", [4, N], F32, kind="ExternalInput").ap()
    qT_d = nc.dram_tensor("qT", [4, NQ], F32, kind="ExternalInput").ap()
    ids_d = nc.dram_tensor("ids", [NQ, NSEL], U16, kind="ExternalOutput").ap()
    with tile.TileContext(nc) as tc:
        with (
            tc.tile_pool(name="tabs", bufs=1) as tabs,
            tc.tile_pool(name="psum", bufs=8, space="PSUM") as pp,
            tc.tile_pool(name="work", bufs=2) as wp,
            tc.tile_pool(name="small", bufs=3) as sp,
        ):
            xyzT_sb = tabs.tile([4, N], F32)
            qT_sb = tabs.tile([4, NQ], F32)
            nc.sync.dma_start(out=xyzT_sb[:], in_=xyzT_d[:])
            nc.sync.dma_start(out=qT_sb[:], in_=qT_d[:])
            for i in range(repeat * NBLK):
                ib = i % NBLK
                lhsT = qT_sb[:, ib * 128:(ib + 1) * 128]
                s_sb = wp.tile([128, N], F32, tag="s_sb", name=f"s_{i}")
                c16 = wp.tile([128, NCH], F32, tag="c16", name=f"c16_{i}")
                for n in range(8):
                    ps = pp.tile([128, 512], F32, tag="ps", name=f"ps_{i}_{n}")
                    nc.tensor.matmul(ps[:], lhsT,
                                     xyzT_sb[:, n * 512:(n + 1) * 512],
                                     start=True, stop=True)
                    nc.scalar.activation(s_sb[:, n * 512:(n + 1) * 512], ps[:],
                                         AF.Copy)
                    nc.vector.tensor_reduce(
                        c16[:, n * 32:(n + 1) * 32],
                        s_sb[:, n * 512:(n + 1) * 512]
                        .rearrange("p (c w) -> p c w", w=CH),
                        axis=AX.X, op=OP.max)
                ids = sp.tile([128, NSEL], U16, tag="ids", name=f"ids_{i}")
                _rounds(nc, sp, c16[:], ids, "a")
                nc.sync.dma_start(out=ids_d[ib * 128:(ib + 1) * 128, :], in_=ids[:])
    nc.compile()
    return nc


def _build_l2a(repeat=1):
    nc = bacc.Bacc("TRN2", target_bir_lowering=False, debug=False,
                   num_devices=NCORES)
    g_d = nc.dram_tensor("g", [NQ, 3 * W], F32, kind="ExternalInput").ap()
    q_d = nc.dram_tensor("q", [NQ, 3], F32, kind="ExternalInput").ap()
    loc_d = nc.dram_tensor("loc", [NQ, NSEL], U16, kind="ExternalOutput").ap()
    with tile.TileContext(nc) as tc:
        with (
            tc.tile_pool(name="work", bufs=3) as wp,
            tc.tile_pool(name="small", bufs=3) as sp,
        ):
            for i in range(repeat * NBLK):
                ib = i % NBLK
                sl = slice(ib * 128, (ib + 1) * 128)
                gt = wp.tile([128, 3 * W], F32, tag="gt", name=f"gt_{i}")
                qx = sp.tile([128, 3], F32, tag="qx", name=f"qx_{i}")
                nc.sync.dma_start(out=gt[:], in_=g_d[sl, :])
                nc.sync.dma_start(out=qx[:], in_=q_d[sl, :])
                nq = sp.tile([128, 3], F32, tag="nq", name=f"nq_{i}")
                nc.vector.tensor_scalar(nq[:], qx[:], -1.0, scalar2=None,
                                        op0=OP.mult)
                d = wp.tile([128, 3, W], F32, tag="d", name=f"d_{i}")
                for c in range(3):
                    nc.scalar.activation(d[:, c, :], gt[:, c * W:(c + 1) * W],
                                         AF.Identity, bias=nq[:, c:c + 1],
                                         scale=1.0)
                nsq = wp.tile([128, 3, W], F32, tag="nsq", name=f"nsq_{i}")
                for c in range(3):
                    nc.vector.scalar_tensor_tensor(
                        nsq[:, c, :], in0=d[:, c, :], scalar=-1.0, in1=d[:, c, :],
                        op0=OP.mult, op1=OP.mult)
                nd = wp.tile([128, W], F32, tag="nd", name=f"nd_{i}")
                nc.vector.tensor_tensor(nd[:], nsq[:, 0, :], nsq[:, 1, :], op=OP.add)
                nc.vector.tensor_tensor(nd[:], nd[:], nsq[:, 2, :], op=OP.add)
                loc = sp.tile([128, NSEL], U16, tag="loc", name=f"loc_{i}")
                _rounds(nc, sp, nd[:], loc, "b")
                nc.sync.dma_start(out=loc_d[sl, :], in_=loc[:])
    nc.compile()
    return nc


def _build_l2b(repeat=1):
    nc = bacc.Bacc("TRN2", target_bir_lowering=False, debug=False,
                   num_devices=NCORES)
    g6_d = nc.dram_tensor("g6", [6, NQ * 8], F32, kind="ExternalInput").ap()
    xq6_d = nc.dram_tensor("xq6", [6, NQ * 8], F32, kind="ExternalInput").ap()
    w1_d = nc.dram_tensor("w1b", [6, 128], F32, kind="ExternalInput").ap()
    w1n_d = nc.dram_tensor("w1nb", [6, 128], F32, kind="ExternalInput").ap()
    w2_d = nc.dram_tensor("w2b", [128, 128], F32, kind="ExternalInput").ap()
    w3_d = nc.dram_tensor("w3b", [128, 128], F32, kind="ExternalInput").ap()
    eye_d = nc.dram_tensor("eye", [128, 128], F32, kind="ExternalInput").ap()
    out_d = nc.dram_tensor("out", [NQ, C], F32, kind="ExternalOutput").ap()
    with tile.TileContext(nc) as tc:
        with (
            tc.tile_pool(name="tabs", bufs=1) as tabs,
            tc.tile_pool(name="psum", bufs=2, space="PSUM") as pp,
            tc.tile_pool(name="work", bufs=2) as wp,
            tc.tile_pool(name="small", bufs=3) as sp,
        ):
            w1_sb = tabs.tile([6, 128], F32)
            w1n_sb = tabs.tile([6, 128], F32)
            w2_sb = tabs.tile([128, 128], F32)
            w3_sb = tabs.tile([128, 128], F32)
            eye_sb = tabs.tile([128, 128], F32)
            g6_sb = tabs.tile([6, NQ * 8], F32)
            xq6_sb = tabs.tile([6, NQ * 8], F32)
            for sb, dd in ((w1_sb, w1_d), (w1n_sb, w1n_d), (w2_sb, w2_d),
                           (w3_sb, w3_d), (eye_sb, eye_d), (g6_sb, g6_d),
                           (xq6_sb, xq6_d)):
                nc.sync.dma_start(out=sb[:], in_=dd[:])
            for i in range(repeat * NBLK):
                ib = i % NBLK
                mx = sp.tile([128, 128], F32, tag="mx", name=f"mx_{i}")
                for t in range(2):
                    cs = slice(ib * 1024 + t * 512, ib * 1024 + (t + 1) * 512)
                    ps1 = pp.tile([128, 512], F32, tag="ps1", name=f"ps1_{i}_{t}")
                    nc.tensor.matmul(ps1[:], w1_sb[:], g6_sb[:, cs],
                                     start=True, stop=False)
                    nc.tensor.matmul(ps1[:], w1n_sb[:], xq6_sb[:, cs],
                                     start=False, stop=True)
                    h1 = wp.tile([128, 512], F32, tag="h1", name=f"h1_{i}_{t}")
                    nc.scalar.activation(h1[:], ps1[:], AF.Relu)
                    ps2 = pp.tile([128, 512], F32, tag="ps2", name=f"ps2_{i}_{t}")
                    nc.tensor.matmul(ps2[:], w2_sb[:], h1[:], start=True, stop=True)
                    h2 = wp.tile([128, 512], F32, tag="h2", name=f"h2_{i}_{t}")
                    nc.scalar.activation(h2[:], ps2[:], AF.Relu)
                    ps3 = pp.tile([128, 512], F32, tag="ps3", name=f"ps3_{i}_{t}")
                    nc.tensor.matmul(ps3[:], w3_sb[:], h2[:], start=True, stop=True)
                    nc.vector.tensor_reduce(
                        mx[:, t * 64:(t + 1) * 64],
                        ps3[:].rearrange("m (q p) -> m q p", p=8),
                        axis=AX.X, op=OP.max)
                pst = pp.tile([128, 128], F32, tag="pst", name=f"pst_{i}")
                nc.tensor.transpose(pst[:], mx[:], eye_sb[:])
                mxT = sp.tile([128, 128], F32, tag="mxT", name=f"mxT_{i}")
                nc.scalar.activation(mxT[:], pst[:], AF.Copy)
                fin = sp.tile([128, 64], F32, tag="fin", name=f"fin_{i}")
                nc.vector.tensor_tensor(fin[:], mxT[:, 0:64], mxT[:, 64:128],
                                        op=OP.max)
                nc.sync.dma_start(out=out_d[ib * 128:(ib + 1) * 128, :], in_=fin[:])
    nc.compile()
    return nc


class _Executor:
    """Cached multi-core PJRT executor for one prebuilt Bass program."""

    def __init__(self, nc):
        install_neuronx_cc_hook()
        self.nc = nc
        part_name = nc.partition_id_tensor.name if nc.partition_id_tensor else None
        in_names, out_names, out_avals, zero_outs = [], [], [], []
        for alloc in nc.m.functions[0].allocations:
            if not isinstance(alloc, mybir.MemoryLocationSet):
                continue
            name = alloc.memorylocations[0].name
            if alloc.kind == "ExternalInput":
                if name != part_name:
                    in_names.append(name)
            elif alloc.kind == "ExternalOutput":
                shape = tuple(alloc.tensor_shape)
                dtype = mybir.dt.np(alloc.dtype)
                out_names.append(name)
                out_avals.append(jax.core.ShapedArray(shape, dtype))
                zero_outs.append(_np.zeros(shape, dtype))
        self.in_names, self.out_names = in_names, out_names
        self.out_avals, self.zero_outs = out_avals, zero_outs
        n_params = len(in_names)
        all_names = in_names + out_names
        if part_name is not None:
            all_names = all_names + [part_name]

        def _body(*args):
            operands = list(args)
            if part_name is not None:
                operands.append(bass2jax.partition_id_tensor())
            return tuple(_bass_exec_p.bind(
                *operands,
                out_avals=tuple(out_avals),
                in_names=tuple(all_names),
                out_names=tuple(out_names),
                lowering_input_output_aliases=(),
                sim_require_finite=True,
                sim_require_nnan=True,
                nc=nc,
            ))

        devices = jax.devices()[:NCORES]
        mesh = Mesh(_np.asarray(devices), ("core",))
        n_outs = len(out_names)
        self._fn = jax.jit(
            shard_map(_body, mesh=mesh,
                      in_specs=(PartitionSpec("core"),) * (n_params + n_outs),
                      out_specs=(PartitionSpec("core"),) * n_outs,
                      check_rep=False),
            donate_argnums=tuple(range(n_params, n_params + n_outs)),
            keep_unused=True,
        )

    def prepare(self, in_maps):
        n = NCORES
        return [
            _np.concatenate([_np.asarray(in_maps[c][name]) for c in range(n)], axis=0)
            for name in self.in_names
        ]

    def run_prepared(self, concat_in):
        n = NCORES
        concat_zeros = [_np.zeros((n * z.shape[0], *z.shape[1:]), z.dtype)
                        for z in self.zero_outs]
        return self._fn(*concat_in, *concat_zeros)

    def __call__(self, in_maps):
        n = NCORES
        outs = self.run_prepared(self.prepare(in_maps))
        outs = [_np.asarray(o) for o in outs]
        return [
            {name: outs[i].reshape(n, *self.out_avals[i].shape)[c]
             for i, name in enumerate(self.out_names)}
            for c in range(n)
        ]


def _get_progs():
    if "l1" not in _progs:
        _progs["l1"] = _Executor(_build_l1())
        _progs["l2a"] = _Executor(_build_l2a())
        _progs["l2b"] = _Executor(_build_l2b())
    return _progs["l1"], _progs["l2a"], _progs["l2b"]


def kernel(xyz, w1, w2, w3, k):
    xyz = np.asarray(xyz, dtype=np.float32)
    w1 = np.asarray(w1, dtype=np.float32)
    w2 = np.asarray(w2, dtype=np.float32)
    w3 = np.asarray(w3, dtype=np.float32)
    assert int(k) == K and xyz.shape == (B, N, 3)
    l1, l2a, l2b = _get_progs()
    cores = list(range(NCORES))

    # ---- L1: coarse chunk selection -------------------------------------
    xyzT_b = []
    for b in range(B):
        X = xyz[b]
        sq = (X[:, 0] ** 2 + X[:, 1] ** 2 + X[:, 2] ** 2).astype(np.float32)
        xyzT_b.append(np.stack([2 * X[:, 0], 2 * X[:, 1], 2 * X[:, 2], sq])
                      .astype(np.float32))
    in1 = []
    for c in cores:
        b, h = c // 2, c % 2
        Q = xyz[b, h * NQ:(h + 1) * NQ]
        qT = np.stack([Q[:, 0], Q[:, 1], Q[:, 2],
                       -np.ones(NQ, np.float32)]).astype(np.float32)
        in1.append({"xyzT": xyzT_b[b], "qT": qT})
    r1 = l1(in1)

    # ---- host glue: superset gather ------------------------------------
    sup = []   # per-core (NQ, W) global candidate ids
    in2 = []
    for c in cores:
        b, h = c // 2, c % 2
        ids = r1[c]["ids"].astype(np.int64)            # (NQ, 24)
        s = (ids[:, :, None] * CH + np.arange(CH)[None, None, :]).reshape(NQ, W)
        sup.append(s)
        g = xyz[b][s]                                          # (NQ, W, 3)
        g3 = np.ascontiguousarray(g.transpose(0, 2, 1)).reshape(NQ, 3 * W)
        q3 = np.ascontiguousarray(xyz[b, h * NQ:(h + 1) * NQ])
        in2.append({"g": g3.astype(np.float32), "q": q3.astype(np.float32)})
    r2 = l2a(in2)

    # ---- host glue: final-16 gather ------------------------------------
    w1blkT = np.zeros((6, 128), np.float32)
    w1blkT[0:3, 0:64] = w1.T
    w1blkT[3:6, 64:128] = w1.T
    w2blkT = np.zeros((128, 128), np.float32)
    w2blkT[0:64, 0:64] = w2.T
    w2blkT[64:128, 64:128] = w2.T
    w3blkT = np.zeros((128, 128), np.float32)
    w3blkT[0:64, 0:64] = w3.T
    w3blkT[64:128, 64:128] = w3.T
    eye = np.eye(128, dtype=np.float32)
    in3 = []
    for c in cores:
        b, h = c // 2, c % 2
        loc = r2[c]["loc"].astype(np.int64)            # (NQ, 24)
        glob = np.take_along_axis(sup[c], loc[:, 1:KK], axis=1)  # (NQ, 16)
        g16 = xyz[b][glob]                                     # (NQ, 16, 3)
        gA, gB = g16[:, 0::2, :], g16[:, 1::2, :]
        g6 = np.concatenate([gA, gB], axis=2)                  # (NQ, 8, 6)
        g6 = np.ascontiguousarray(g6.transpose(2, 0, 1)).reshape(6, NQ * 8)
        q = xyz[b, h * NQ:(h + 1) * NQ]
        xq6 = np.repeat(np.concatenate([q, q], axis=1)[:, None, :], 8, axis=1)
        xq6 = np.ascontiguousarray(xq6.transpose(2, 0, 1)).reshape(6, NQ * 8)
        in3.append({"g6": g6.astype(np.float32), "xq6": xq6.astype(np.float32),
                    "w1b": w1blkT, "w1nb": -w1blkT, "w2b": w2blkT,
                    "w3b": w3blkT, "eye": eye})
    r3 = l2b(in3)

    out = np.zeros((B, C, N), np.float32)
    for c in cores:
        b, h = c // 2, c % 2
        out[b, :, h * NQ:(h + 1) * NQ] = r3[c]["out"].T
    return out


# revision 14
# speedup vs baseline: 1.0275x; 1.0275x over previous
"""kNN (k=16) + grouped 3->64->64->64 MLP + neighbor max-pool on 8 TRN2 cores.

Pipeline (device does all O(N^2) compute, selection, and MLP flops):
  L1 : S[q,j] = 2<xq,xj> - |xj|^2 on PE (self is always row max); chunk-16 max;
       top-24 chunk ids per query via max8/max_index/match_replace rounds.
  host: gather the 24*16=384 candidate coords per query (index routing only).
  L2A: exact squared dists in reference fp32 arithmetic on the 384-wide
       compacted domain; exact top-17 (slot 0 = self) -> local indices.
  host: map local->global indices, gather the 16 neighbor coords.
  L2B: relative coords via matmul-folded subtract, packed 2-point 3-layer MLP
       on PE, max-pool over the 16 neighbors.

Sharding: core c handles batch c//2, query half c%2 (2048 queries each).
"""
import sys
import numpy as np

sys.path.insert(0, "/opt/trn_rl_repo")

import jax
import numpy as _np
from jax.sharding import Mesh, PartitionSpec
from jax.experimental.shard_map import shard_map

import concourse.bacc as bacc
import concourse.mybir as mybir
import concourse.tile as tile
from concourse import bass2jax
from concourse.bass2jax import _bass_exec_p, install_neuronx_cc_hook

F32 = mybir.dt.float32
U16 = mybir.dt.uint16
AX = mybir.AxisListType
OP = mybir.AluOpType
AF = mybir.ActivationFunctionType

B, N, C, K = 4, 4096, 64, 16
KK = K + 1              # 17
CH = 16                 # chunk size for the selection hierarchy
NCH = N // CH           # 256
NSEL = 24               # chunks kept per query (>= 17 guarantee + tie slack)
W = NSEL * CH           # 384 candidate superset per query
NQ = 2048               # queries per core
NBLK = NQ // 128        # 16
NEG = -1.0e30
NCORES = 8

_progs = {}


def _rounds(nc, sp, vals, out_ids, tag):
    """3x (max8 -> max_index -> match_replace) producing 24 ids, mutating vals."""
    for r in range(3):
        m8 = sp.tile([128, 8], F32, tag=f"m8{tag}", name=f"m8{tag}_{r}_{id(vals)}")
        nc.vector.max(out=m8[:], in_=vals)
        nc.vector.max_index(out=out_ids[:, r * 8:(r + 1) * 8], in_max=m8[:],
                            in_values=vals)
        if r < 2:
            nc.vector.match_replace(out=vals, in_to_replace=m8[:], in_values=vals,
                                    imm_value=NEG)


def _build_l1(repeat=1):
    nc = bacc.Bacc("TRN2", target_bir_lowering=False, debug=False,
                   num_devices=NCORES)
    xyzT_d = nc.dram_tensor("xyzT", [4, N], F32, kind="ExternalInput").ap()
    qT_d = nc.dram_tensor("qT", [4, NQ], F32, kind="ExternalInput").ap()
    ids_d = nc.dram_tensor("ids", [NQ, NSEL], U16, kind="ExternalOutput").ap()
    with tile.TileContext(nc) as tc:
        with (
            tc.tile_pool(name="tabs", bufs=1) as tabs,
            tc.tile_pool(name="psum", bufs=8, space="PSUM") as pp,
            tc.tile_pool(name="work", bufs=3) as wp,
            tc.tile_pool(name="small", bufs=4) as sp,
        ):
            xyzT_sb = tabs.tile([4, N], F32)
            qT_sb = tabs.tile([4, NQ], F32)
            nc.sync.dma_start(out=xyzT_sb[:], in_=xyzT_d[:])
            nc.sync.dma_start(out=qT_sb[:], in_=qT_d[:])
            for i in range(repeat * NBLK):
                ib = i % NBLK
                lhsT = qT_sb[:, ib * 128:(ib + 1) * 128]
                s_sb = wp.tile([128, N], F32, tag="s_sb", name=f"s_{i}")
                c16 = wp.tile([128, NCH], F32, tag="c16", name=f"c16_{i}")
                for n in range(8):
                    ps = pp.tile([128, 512], F32, tag="ps", name=f"ps_{i}_{n}")
                    nc.tensor.matmul(ps[:], lhsT,
                                     xyzT_sb[:, n * 512:(n + 1) * 512],
                                     start=True, stop=True)
                    nc.scalar.activation(s_sb[:, n * 512:(n + 1) * 512], ps[:],
                                         AF.Copy)
                    nc.vector.tensor_reduce(
                        c16[:, n * 32:(n + 1) * 32],
                        s_sb[:, n * 512:(n + 1) * 512]
                        .rearrange("p (c w) -> p c w", w=CH),
                        axis=AX.X, op=OP.max)
                ids = sp.tile([128, NSEL], U16, tag="ids", name=f"ids_{i}")
                _rounds(nc, sp, c16[:], ids, "a")
                nc.sync.dma_start(out=ids_d[ib * 128:(ib + 1) * 128, :], in_=ids[:])
    nc.compile()
    return nc


def _build_l2a(repeat=1):
    nc = bacc.Bacc("TRN2", target_bir_lowering=False, debug=False,
                   num_devices=NCORES)
    g_d = nc.dram_tensor("g", [NQ, 3 * W], F32, kind="ExternalInput").ap()
    q_d = nc.dram_tensor("q", [NQ, 3], F32, kind="ExternalInput").ap()
    loc_d = nc.dram_tensor("loc", [NQ, NSEL], U16, kind="ExternalOutput").ap()
    with tile.TileContext(nc) as tc:
        with (
            tc.tile_pool(name="work", bufs=3) as wp,
            tc.tile_pool(name="small", bufs=3) as sp,
        ):
            for i in range(repeat * NBLK):
                ib = i % NBLK
                sl = slice(ib * 128, (ib + 1) * 128)
                gt = wp.tile([128, 3 * W], F32, tag="gt", name=f"gt_{i}")
                qx = sp.tile([128, 3], F32, tag="qx", name=f"qx_{i}")
                nc.sync.dma_start(out=gt[:], in_=g_d[sl, :])
                nc.sync.dma_start(out=qx[:], in_=q_d[sl, :])
                nq = sp.tile([128, 3], F32, tag="nq", name=f"nq_{i}")
                nc.vector.tensor_scalar(nq[:], qx[:], -1.0, scalar2=None,
                                        op0=OP.mult)
                d = wp.tile([128, 3, W], F32, tag="d", name=f"d_{i}")
                for c in range(3):
                    nc.scalar.activation(d[:, c, :], gt[:, c * W:(c + 1) * W],
                                         AF.Identity, bias=nq[:, c:c + 1],
                                         scale=1.0)
                nsq = wp.tile([128, 3, W], F32, tag="nsq", name=f"nsq_{i}")
                for c in range(3):
                    nc.vector.scalar_tensor_tensor(
                        nsq[:, c, :], in0=d[:, c, :], scalar=-1.0, in1=d[:, c, :],
                        op0=OP.mult, op1=OP.mult)
                nd = wp.tile([128, W], F32, tag="nd", name=f"nd_{i}")
                nc.gpsimd.tensor_tensor(nd[:], nsq[:, 0, :], nsq[:, 1, :], op=OP.add)
                nc.gpsimd.tensor_tensor(nd[:], nd[:], nsq[:, 2, :], op=OP.add)
                loc = sp.tile([128, NSEL], U16, tag="loc", name=f"loc_{i}")
                _rounds(nc, sp, nd[:], loc, "b")
                nc.sync.dma_start(out=loc_d[sl, :], in_=loc[:])
    nc.compile()
    return nc


def _build_l2b(repeat=1):
    nc = bacc.Bacc("TRN2", target_bir_lowering=False, debug=False,
                   num_devices=NCORES)
    g6_d = nc.dram_tensor("g6", [6, NQ * 8], F32, kind="ExternalInput").ap()
    xq6_d = nc.dram_tensor("xq6", [6, NQ * 8], F32, kind="ExternalInput").ap()
    w1_d = nc.dram_tensor("w1b", [6, 128], F32, kind="ExternalInput").ap()
    w1n_d = nc.dram_tensor("w1nb", [6, 128], F32, kind="ExternalInput").ap()
    w2_d = nc.dram_tensor("w2b", [128, 128], F32, kind="ExternalInput").ap()
    w3_d = nc.dram_tensor("w3b", [128, 128], F32, kind="ExternalInput").ap()
    eye_d = nc.dram_tensor("eye", [128, 128], F32, kind="ExternalInput").ap()
    out_d = nc.dram_tensor("out", [NQ, C], F32, kind="ExternalOutput").ap()
    with tile.TileContext(nc) as tc:
        with (
            tc.tile_pool(name="tabs", bufs=1) as tabs,
            tc.tile_pool(name="psum", bufs=2, space="PSUM") as pp,
            tc.tile_pool(name="psum3", bufs=3, space="PSUM") as pp3,
            tc.tile_pool(name="psumT", bufs=1, space="PSUM") as ppt,
            tc.tile_pool(name="work", bufs=4) as wp,
            tc.tile_pool(name="small", bufs=4) as sp,
        ):
            w1_sb = tabs.tile([6, 128], F32)
            w1n_sb = tabs.tile([6, 128], F32)
            w2_sb = tabs.tile([128, 128], F32)
            w3_sb = tabs.tile([128, 128], F32)
            eye_sb = tabs.tile([128, 128], F32)
            g6_sb = tabs.tile([6, NQ * 8], F32)
            xq6_sb = tabs.tile([6, NQ * 8], F32)
            for sb, dd in ((w1_sb, w1_d), (w1n_sb, w1n_d), (w2_sb, w2_d),
                           (w3_sb, w3_d), (eye_sb, eye_d), (g6_sb, g6_d),
                           (xq6_sb, xq6_d)):
                nc.sync.dma_start(out=sb[:], in_=dd[:])
            for i in range(repeat * NBLK):
                ib = i % NBLK
                mx = sp.tile([128, 128], F32, tag="mx", name=f"mx_{i}")
                for t in range(2):
                    cs = slice(ib * 1024 + t * 512, ib * 1024 + (t + 1) * 512)
                    ps1 = pp.tile([128, 512], F32, tag="ps1", name=f"ps1_{i}_{t}")
                    nc.tensor.matmul(ps1[:], w1_sb[:], g6_sb[:, cs],
                                     start=True, stop=False)
                    nc.tensor.matmul(ps1[:], w1n_sb[:], xq6_sb[:, cs],
                                     start=False, stop=True)
                    h1 = wp.tile([128, 512], F32, tag="h1", name=f"h1_{i}_{t}")
                    nc.scalar.activation(h1[:], ps1[:], AF.Relu)
                    ps2 = pp.tile([128, 512], F32, tag="ps2", name=f"ps2_{i}_{t}")
                    nc.tensor.matmul(ps2[:], w2_sb[:], h1[:], start=True, stop=True)
                    h2 = wp.tile([128, 512], F32, tag="h2", name=f"h2_{i}_{t}")
                    nc.scalar.activation(h2[:], ps2[:], AF.Relu)
                    ps3 = pp3.tile([128, 512], F32, tag="ps3", name=f"ps3_{i}_{t}")
                    nc.tensor.matmul(ps3[:], w3_sb[:], h2[:], start=True, stop=True)
                    nc.vector.tensor_reduce(
                        mx[:, t * 64:(t + 1) * 64],
                        ps3[:].rearrange("m (q p) -> m q p", p=8),
                        axis=AX.X, op=OP.max)
                pst = ppt.tile([128, 128], F32, tag="pst", name=f"pst_{i}")
                nc.tensor.transpose(pst[:], mx[:], eye_sb[:])
                mxT = sp.tile([128, 128], F32, tag="mxT", name=f"mxT_{i}")
                nc.scalar.activation(mxT[:], pst[:], AF.Copy)
                fin = sp.tile([128, 64], F32, tag="fin", name=f"fin_{i}")
                nc.vector.tensor_tensor(fin[:], mxT[:, 0:64], mxT[:, 64:128],
                                        op=OP.max)
                nc.sync.dma_start(out=out_d[ib * 128:(ib + 1) * 128, :], in_=fin[:])
    nc.compile()
    return nc


class _Executor:
    """Cached multi-core PJRT executor for one prebuilt Bass program."""

    def __init__(self, nc):
        install_neuronx_cc_hook()
        self.nc = nc
        part_name = nc.partition_id_tensor.name if nc.partition_id_tensor else None
        in_names, out_names, out_avals, zero_outs = [], [], [], []
        for alloc in nc.m.functions[0].allocations:
            if not isinstance(alloc, mybir.MemoryLocationSet):
                continue
            name = alloc.memorylocations[0].name
            if alloc.kind == "ExternalInput":
                if name != part_name:
                    in_names.append(name)
            elif alloc.kind == "ExternalOutput":
                shape = tuple(alloc.tensor_shape)
                dtype = mybir.dt.np(alloc.dtype)
                out_names.append(name)
                out_avals.append(jax.core.ShapedArray(shape, dtype))
                zero_outs.append(_np.zeros(shape, dtype))
        self.in_names, self.out_names = in_names, out_names
        self.out_avals, self.zero_outs = out_avals, zero_outs
        n_params = len(in_names)
        all_names = in_names + out_names
        if part_name is not None:
            all_names = all_names + [part_name]

        def _body(*args):
            operands = list(args)
            if part_name is not None:
                operands.append(bass2jax.partition_id_tensor())
            return tuple(_bass_exec_p.bind(
                *operands,
                out_avals=tuple(out_avals),
                in_names=tuple(all_names),
                out_names=tuple(out_names),
                lowering_input_output_aliases=(),
                sim_require_finite=True,
                sim_require_nnan=True,
                nc=nc,
            ))

        devices = jax.devices()[:NCORES]
        mesh = Mesh(_np.asarray(devices), ("core",))
        n_outs = len(out_names)
        self._fn = jax.jit(
            shard_map(_body, mesh=mesh,
                      in_specs=(PartitionSpec("core"),) * (n_params + n_outs),
                      out_specs=(PartitionSpec("core"),) * n_outs,
                      check_rep=False),
            donate_argnums=tuple(range(n_params, n_params + n_outs)),
            keep_unused=True,
        )

    def prepare(self, in_maps):
        n = NCORES
        return [
            _np.concatenate([_np.asarray(in_maps[c][name]) for c in range(n)], axis=0)
            for name in self.in_names
        ]

    def run_prepared(self, concat_in):
        n = NCORES
        concat_zeros = [_np.zeros((n * z.shape[0], *z.shape[1:]), z.dtype)
                        for z in self.zero_outs]
        return self._fn(*concat_in, *concat_zeros)

    def __call__(self, in_maps):
        n = NCORES
        outs = self.run_prepared(self.prepare(in_maps))
        outs = [_np.asarray(o) for o in outs]
        return [
            {name: outs[i].reshape(n, *self.out_avals[i].shape)[c]
             for i, name in enumerate(self.out_names)}
            for c in range(n)
        ]


def _get_progs():
    if "l1" not in _progs:
        _progs["l1"] = _Executor(_build_l1())
        _progs["l2a"] = _Executor(_build_l2a())
        _progs["l2b"] = _Executor(_build_l2b())
    return _progs["l1"], _progs["l2a"], _progs["l2b"]


def kernel(xyz, w1, w2, w3, k):
    xyz = np.asarray(xyz, dtype=np.float32)
    w1 = np.asarray(w1, dtype=np.float32)
    w2 = np.asarray(w2, dtype=np.float32)
    w3 = np.asarray(w3, dtype=np.float32)
    assert int(k) == K and xyz.shape == (B, N, 3)
    l1, l2a, l2b = _get_progs()
    cores = list(range(NCORES))

    # ---- L1: coarse chunk selection -------------------------------------
    xyzT_b = []
    for b in range(B):
        X = xyz[b]
        sq = (X[:, 0] ** 2 + X[:, 1] ** 2 + X[:, 2] ** 2).astype(np.float32)
        xyzT_b.append(np.stack([2 * X[:, 0], 2 * X[:, 1], 2 * X[:, 2], sq])
                      .astype(np.float32))
    in1 = []
    for c in cores:
        b, h = c // 2, c % 2
        Q = xyz[b, h * NQ:(h + 1) * NQ]
        qT = np.stack([Q[:, 0], Q[:, 1], Q[:, 2],
                       -np.ones(NQ, np.float32)]).astype(np.float32)
        in1.append({"xyzT": xyzT_b[b], "qT": qT})
    r1 = l1(in1)

    # ---- host glue: superset gather ------------------------------------
    sup = []   # per-core (NQ, W) global candidate ids
    in2 = []
    for c in cores:
        b, h = c // 2, c % 2
        ids = r1[c]["ids"].astype(np.int64)            # (NQ, 24)
        s = (ids[:, :, None] * CH + np.arange(CH)[None, None, :]).reshape(NQ, W)
        sup.append(s)
        g = xyz[b][s]                                          # (NQ, W, 3)
        g3 = np.ascontiguousarray(g.transpose(0, 2, 1)).reshape(NQ, 3 * W)
        q3 = np.ascontiguousarray(xyz[b, h * NQ:(h + 1) * NQ])
        in2.append({"g": g3.astype(np.float32), "q": q3.astype(np.float32)})
    r2 = l2a(in2)

    # ---- host glue: final-16 gather ------------------------------------
    w1blkT = np.zeros((6, 128), np.float32)
    w1blkT[0:3, 0:64] = w1.T
    w1blkT[3:6, 64:128] = w1.T
    w2blkT = np.zeros((128, 128), np.float32)
    w2blkT[0:64, 0:64] = w2.T
    w2blkT[64:128, 64:128] = w2.T
    w3blkT = np.zeros((128, 128), np.float32)
    w3blkT[0:64, 0:64] = w3.T
    w3blkT[64:128, 64:128] = w3.T
    eye = np.eye(128, dtype=np.float32)
    in3 = []
    for c in cores:
        b, h = c // 2, c % 2
        loc = r2[c]["loc"].astype(np.int64)            # (NQ, 24)
        glob = np.take_along_axis(sup[c], loc[:, 1:KK], axis=1)  # (NQ, 16)
        g16 = xyz[b][glob]                                     # (NQ, 16, 3)
        gA, gB = g16[:, 0::2, :], g16[:, 1::2, :]
        g6 = np.concatenate([gA, gB], axis=2)                  # (NQ, 8, 6)
        g6 = np.ascontiguousarray(g6.transpose(2, 0, 1)).reshape(6, NQ * 8)
        q = xyz[b, h * NQ:(h + 1) * NQ]
        xq6 = np.repeat(np.concatenate([q, q], axis=1)[:, None, :], 8, axis=1)
        xq6 = np.ascontiguousarray(xq6.transpose(2, 0, 1)).reshape(6, NQ * 8)
        in3.append({"g6": g6.astype(np.float32), "xq6": xq6.astype(np.float32),
                    "w1b": w1blkT, "w1nb": -w1blkT, "w2b": w2blkT,
                    "w3b": w3blkT, "eye": eye})
    r3 = l2b(in3)

    out = np.zeros((B, C, N), np.float32)
    for c in cores:
        b, h = c // 2, c % 2
        out[b, :, h * NQ:(h + 1) * NQ] = r3[c]["out"].T
    return out
